# revision 7
# baseline (speedup 1.0000x reference)
"""Trainium2 Bass kernel for a 2-layer LSTM regressor (B=128, T=4096, D=64, H=512).

Strategy (v3): sequence-parallel across the 8 cores (each core runs a 512-step
chunk with a 16-step burn-in prefix; the LSTM state contracts so initial-state
error decays to ~1e-11 by step 16). Full batch B=128 is the matmul moving
operand on every core.

v3 changes vs v2 (which measured 7.89 ms, ~97% PE-busy):
  - L2's 16 bias matmuls/step removed: the per-gate-unit bias is pre-written
    into the PSUM gate tiles by the DVE (tensor_copy of a pre-broadcast f32
    bias tile) and the gate matmuls run with start=False, accumulating onto it
    (has_written bits stay set after each PSUM buffer's first real group).
  - Gate quarters (i, f, g, o) in separate 1-bank PSUM tiles with per-quarter
    activations issued in dependency order, so the o-gate sigmoid + h-multiply
    are the only ops after the last gate matmul: recurrence chain tail drops
    from ~6 us to ~1.5 us and the per-step chain stalls disappear.
  - h1/h2 histories are 16-slot rings indexed (global_step % 16) - statically
    resolvable inside the hardware loop (32 steps/iteration) - removing the
    per-block carry copies from the critical path.
  - y emitted per 4-step block with FD=512 matmuls (4 instead of 16 per block)
    plus a DVE tensor_scalar bias add (bout matmul removed).
  - x DMAs for all 8 halves issued at the iteration top (bufs=10) so the SP
    engine prefetches a full iteration ahead; loop-head PE stalls (~8 us x 32)
    disappear.
"""
import sys
sys.path.insert(0, "/opt/trn_rl_repo")
import numpy as np
import ml_dtypes

import concourse.bass as bass
import concourse.bacc as bacc
import concourse.tile as tile
from concourse import mybir, bass_utils
from concourse.bass import ds, ts

F32 = mybir.dt.float32
BF16 = mybir.dt.bfloat16
AF = mybir.ActivationFunctionType
BF = ml_dtypes.bfloat16

B, T, D_IN = 128, 4096, 64
H, D_OUT = 512, 64
NC = 8
SB = 4           # steps per block
W_BURN = 16      # burn-in steps
UNROLL = 4       # L1/L2 block pairs ("halves") per loop iteration is 2*UNROLL
RING = 16        # h-history ring slots; must divide 4*2*UNROLL


# ---------------------------------------------------------------- host prep
def _prep_whh(w):
    """[2048, 512] -> [128, 16*4*128] bf16 lhsT tiles, col=(m*4+k)*128+j.
    Gate order is torch order (i, f, g, o): m0-3=i, m4-7=f, m8-11=g, m12-15=o."""
    t = w.reshape(16, 128, 4, 128)  # [m, j, k, p]
    t = t.transpose(3, 0, 2, 1)  # [p, m, k, j]
    return np.ascontiguousarray(t.reshape(128, 16 * 4 * 128)).astype(BF)


def _prep_wih1(w_ih1, b1):
    """[2048, 64] + bias -> [128, 2048] bf16 (rows 64/65 = bias hi/lo, 66+ zero)."""
    top = w_ih1.T  # [64, 2048]
    b = b1.astype(np.float32)
    bhi = b.astype(BF).astype(np.float32)
    blo = b - bhi
    pad = np.zeros((62, 2048), np.float32)
    return np.concatenate([top, bhi[None], blo[None], pad], 0).astype(BF)


def _prep_b2f(b2):
    """bias -> [128, 2048] f32 pre-broadcast: [p, m*128+c] = b2[m*128+p]."""
    v = b2.astype(np.float32).reshape(16, 128)  # [m, p]
    arr = np.broadcast_to(v.T[:, :, None], (128, 16, 128))
    return np.ascontiguousarray(arr.reshape(128, 2048))


def _prep_wout(w_out):
    """[64, 512] -> [128, 4*64]: [p, k*64+d] = w_out[d, k*128+p]."""
    t = w_out.reshape(64, 4, 128).transpose(2, 1, 0)  # [p, k, d]
    return np.ascontiguousarray(t.reshape(128, 256)).astype(BF)


def _prep_x(x_core):
    """[128, T_c, 64] -> blocks [NB, 128, 4*128] bf16 (col = s*128 + b).
    Rows 64/65 = 1.0 (bias), rows 66..127 = 0 (pad to full K=128)."""
    bc, T_c, _ = x_core.shape
    nb = T_c // SB
    arr = x_core.transpose(1, 2, 0).reshape(nb, SB, 64, bc)  # [blk, s, d, b]
    arr = arr.transpose(0, 2, 1, 3)  # [blk, d, s, b]
    ones = np.ones((nb, 2, SB, bc), np.float32)
    pad = np.zeros((nb, 62, SB, bc), np.float32)
    xa = np.concatenate([arr, ones, pad], 1).reshape(nb, 128, SB * bc)
    return xa.astype(BF)


# ---------------------------------------------------------------- program
def build_program(T_c, n_cores=NC):
    NB = T_c // SB
    HB = 2 * UNROLL                      # halves (block pairs) per iteration
    NJ = (NB - 2) // HB                  # loop iterations; L1 blocks 1..HB*NJ
    NTAIL = NB - 1 - HB * NJ             # L1 blocks unrolled in the epilogue
    assert T_c % SB == 0 and NTAIL >= 1 and (SB * HB) % RING == 0
    nc = bacc.Bacc("TRN2", target_bir_lowering=False, debug=False, num_devices=n_cores)

    d = {}
    d["w1hh"] = nc.dram_tensor("w1hh", [128, 8192], BF16, kind="ExternalInput")
    d["w1ih"] = nc.dram_tensor("w1ih", [128, 2048], BF16, kind="ExternalInput")
    d["w2hh"] = nc.dram_tensor("w2hh", [128, 8192], BF16, kind="ExternalInput")
    d["w2ih"] = nc.dram_tensor("w2ih", [128, 8192], BF16, kind="ExternalInput")
    d["b2f"] = nc.dram_tensor("b2f", [128, 2048], F32, kind="ExternalInput")
    d["wout"] = nc.dram_tensor("wout", [128, 256], BF16, kind="ExternalInput")
    d["boutv"] = nc.dram_tensor("boutv", [64, 1], F32, kind="ExternalInput")
    d["x0"] = nc.dram_tensor("x0", [128, SB * 128], BF16, kind="ExternalInput")
    d["xm"] = nc.dram_tensor("xm", [NJ, HB, 128, SB * 128], BF16, kind="ExternalInput")
    d["xtail"] = nc.dram_tensor("xtail", [NTAIL, 128, SB * 128], BF16, kind="ExternalInput")
    # row b+1 = y(4b..4b+3); row 0 is a dummy (pipeline warmup), dropped on host
    d["y"] = nc.dram_tensor("y", [NB + 1, 64, SB * 128], F32, kind="ExternalOutput")

    with tile.TileContext(nc) as tc:
        with tc.tile_pool(name="persist", bufs=1) as pp, \
             tc.tile_pool(name="work", bufs=2) as wp, \
             tc.tile_pool(name="xin", bufs=10) as xp_pool, \
             tc.tile_pool(name="psum", bufs=1, space="PSUM") as psp:

            w1hh = pp.tile([128, 8192], BF16)
            w1ih = pp.tile([128, 2048], BF16)
            w2hh = pp.tile([128, 8192], BF16)
            w2ih = pp.tile([128, 8192], BF16)
            b2f = pp.tile([128, 16, 128], F32)
            wout = pp.tile([128, 256], BF16)
            boutv = pp.tile([64, 1], F32)
            # order: everything the prologue L1 needs first (w1hh, w1ih, x0)
            nc.sync.dma_start(w1hh[:], d["w1hh"].ap())
            nc.sync.dma_start(w1ih[:], d["w1ih"].ap())
            xc0 = xp_pool.tile([128, SB * 128], BF16, tag="xc")
            nc.sync.dma_start(xc0[:], d["x0"].ap())
            for t_, dr in [(w2hh, "w2hh"), (w2ih, "w2ih"), (b2f, "b2f"),
                           (wout, "wout"), (boutv, "boutv")]:
                nc.sync.dma_start(t_[:], d[dr].ap())

            # h histories: ring of RING slots; step g reads slot g%RING,
            # writes (g+1)%RING. Zeros = initial state.
            H1 = pp.tile([128, 4, RING, 128], BF16)
            H2 = pp.tile([128, 4, RING, 128], BF16)
            c1 = pp.tile([128, 4, 128], F32)
            c2 = pp.tile([128, 4, 128], F32)
            nc.vector.memset(H1[:], 0.0)
            nc.vector.memset(H2[:], 0.0)
            nc.vector.memset(c1[:], 0.0)
            nc.vector.memset(c2[:], 0.0)

            def gates_and_state(qt, cst, hdst, lt):
                """Per-quarter activations + cell update. qt = [qi,qf,qg,qo].
                f stays f32 (its error is amplified by 1/(1-f) in the c fixpoint);
                i,g,o,tanh(c) are bf16 (same rounding class as the bf16 h)."""
                ai = wp.tile([128, 4, 128], BF16, tag=f"ai{lt}")
                af = wp.tile([128, 4, 128], F32, tag=f"af{lt}")
                ag = wp.tile([128, 4, 128], BF16, tag=f"ag{lt}")
                ao = wp.tile([128, 4, 128], BF16, tag=f"ao{lt}")
                tmp = wp.tile([128, 4, 128], BF16, tag=f"tmp{lt}")
                tct = wp.tile([128, 4, 128], BF16, tag=f"tct{lt}")
                nc.scalar.activation(ai[:], qt[0][:], AF.Sigmoid)
                nc.scalar.activation(af[:], qt[1][:], AF.Sigmoid)
                nc.scalar.activation(ag[:], qt[2][:], AF.Tanh)
                nc.scalar.activation(ao[:], qt[3][:], AF.Sigmoid)
                nc.vector.tensor_mul(tmp[:], ai[:], ag[:])
                nc.vector.tensor_mul(cst[:], af[:], cst[:])
                nc.vector.tensor_add(cst[:], cst[:], tmp[:])
                nc.scalar.activation(tct[:], cst[:], AF.Tanh)
                nc.vector.tensor_mul(hdst, ao[:], tct[:])

            def l2_alloc_fill():
                """Allocate + bias-fill the 4 L2 gate quarters (issued at step
                start so the DVE fills run ahead of the chain ops)."""
                qt = [psp.tile([128, 4, 128], F32, tag="g", bufs=6, name=f"g2_{q}")
                      for q in range(4)]
                for q in range(4):
                    nc.vector.tensor_copy(qt[q][:], b2f[:, q * 4:(q + 1) * 4, :])
                return qt

            def l1_step(xblk, s, sr, sw):
                xs = xblk[:, s * 128:(s + 1) * 128]
                qt = [psp.tile([128, 4, 128], F32, tag="g", bufs=6, name=f"g1_{q}")
                      for q in range(4)]
                # start=True clears has_written for the WHOLE bank, so only the
                # first matmul per tile starts; later j-regions overwrite via
                # cleared bits, then their hh matmuls accumulate. This also
                # leaves every element's bit set for the next (L2) user.
                for q in range(4):
                    for j in range(4):
                        m = q * 4 + j
                        o = qt[q][:, j, :]
                        nc.tensor.matmul(o, w1ih[:, m * 128:(m + 1) * 128], xs,
                                         start=(j == 0), stop=False)
                        for k in range(4):
                            nc.tensor.matmul(
                                o, w1hh[:, (m * 4 + k) * 128:(m * 4 + k + 1) * 128],
                                H1[:, k, sr, :], start=False, stop=(k == 3))
                gates_and_state(qt, c1, H1[:, :, sw, :], "1")

            def l2_step(qt, sh1, sr2, sw2):
                """Gate matmuls accumulate (start=False) onto the DVE-written
                bias; every PSUM buffer got a start=True L1 group in the
                prologue, so has_written is set and start=False accumulates."""
                for q in range(4):
                    for j in range(4):
                        m = q * 4 + j
                        o = qt[q][:, j, :]
                        for k in range(4):
                            nc.tensor.matmul(
                                o, w2ih[:, (m * 4 + k) * 128:(m * 4 + k + 1) * 128],
                                H1[:, k, sh1, :], start=False, stop=False)
                        for k in range(4):
                            nc.tensor.matmul(
                                o, w2hh[:, (m * 4 + k) * 128:(m * 4 + k + 1) * 128],
                                H2[:, k, sr2, :], start=False, stop=(k == 3))
                gates_and_state(qt, c2, H2[:, :, sw2, :], "2")

            def y_block(y_ap, a):
                """y for one 4-step block from H2 ring slots a..a+3 (mod RING)."""
                yp = psp.tile([64, SB, 128], F32, tag="y", bufs=2)
                n1 = min(SB, RING - a)
                for k in range(4):
                    wk = wout[:, k * 64:(k + 1) * 64]
                    # only the very first matmul starts (bank-wide clear); the
                    # wrapped region's k0 writes via cleared has_written bits
                    nc.tensor.matmul(yp[:, 0:n1, :], wk, H2[:, k, a:a + n1, :],
                                     start=(k == 0), stop=(k == 3))
                    if n1 < SB:
                        nc.tensor.matmul(yp[:, n1:SB, :], wk, H2[:, k, 0:SB - n1, :],
                                         start=False, stop=(k == 3))
                ys = wp.tile([64, SB, 128], F32, tag="ystage")
                nc.vector.tensor_scalar_add(ys[:], yp[:], boutv[:])
                nc.sync.dma_start(y_ap, ys[:])

            def half(xt, y_ap, g0, has_l1=True, has_l2=True):
                """One L1 block (base gstep g0) + the L2 block one behind it
                (base gstep g0-4) + the y block two behind (slots g0-7..g0-4)."""
                y_block(y_ap, (g0 - 7) % RING)
                for s in range(SB):
                    qt2 = l2_alloc_fill() if has_l2 else None
                    if has_l1:
                        l1_step(xt, s, (g0 + s) % RING, (g0 + s + 1) % RING)
                    if has_l2:
                        l2_step(qt2, (g0 - 4 + s + 1) % RING,
                                (g0 - 4 + s) % RING, (g0 - 4 + s + 1) % RING)

            # prologue: L1 block 0 (gsteps 0..3), initial state = ring zeros
            for s in range(SB):
                l1_step(xc0, s, s, s + 1)

            with tc.For_i(0, NJ, 1, staggered_reset=True, hint_engines=(
                    mybir.EngineType.PE, mybir.EngineType.Activation,
                    mybir.EngineType.DVE, mybir.EngineType.SP)) as j:
                xts = []
                for h in range(HB):
                    xt = xp_pool.tile([128, SB * 128], BF16, tag="xc")
                    nc.sync.dma_start(xt[:], d["xm"].ap()[ds(j, 1)][0, h])
                    xts.append(xt)
                yrows = d["y"].ap()[ts(j, HB)]
                for h in range(HB):
                    # L1 block HB*j+1+h -> base gstep 4*(HB*j+1+h); 32j drops mod 16
                    half(xts[h], yrows[h], (4 + 4 * h) % RING)

            # epilogue: NTAIL tail halves + trailing L2 block + final 2 y blocks
            b1 = HB * NJ + 1            # first epilogue L1 block
            xtl = []
            for i in range(NTAIL):
                xt = xp_pool.tile([128, SB * 128], BF16, tag="xc")
                nc.sync.dma_start(xt[:], d["xtail"].ap()[i])
                xtl.append(xt)
            for i in range(NTAIL):
                half(xtl[i], d["y"].ap()[b1 + i - 1], (4 * (b1 + i)) % RING)
            # trailing L2 block (no L1): L1 block would be b1+NTAIL = NB
            g0 = (4 * (b1 + NTAIL)) % RING
            half(None, d["y"].ap()[b1 + NTAIL - 1], g0, has_l1=False)
            # final y block (L2 block NB-1, slots g0-3..g0)
            y_block(d["y"].ap()[NB], (g0 - 3) % RING)

    nc.compile()
    return nc


_CACHE = {}


def _get_program(T_c):
    if T_c not in _CACHE:
        _CACHE[T_c] = build_program(T_c)
    return _CACHE[T_c]


def _host_prep(w_ih1, w_hh1, b_ih1, b_hh1, w_ih2, w_hh2, b_ih2, b_hh2, w_out, b_out):
    return {
        "w1hh": _prep_whh(np.asarray(w_hh1)),
        "w1ih": _prep_wih1(np.asarray(w_ih1), np.asarray(b_ih1) + np.asarray(b_hh1)),
        "w2hh": _prep_whh(np.asarray(w_hh2)),
        "w2ih": _prep_whh(np.asarray(w_ih2)),
        "b2f": _prep_b2f(np.asarray(b_ih2) + np.asarray(b_hh2)),
        "wout": _prep_wout(np.asarray(w_out)),
        "boutv": np.ascontiguousarray(np.asarray(b_out, np.float32)[:, None]),
    }


def _split_x(xa):
    """Split per-block x array into x0/xm/xtail dram layouts."""
    NB = xa.shape[0]
    HB = 2 * UNROLL
    NJ = (NB - 2) // HB
    return {
        "x0": np.ascontiguousarray(xa[0]),
        "xm": np.ascontiguousarray(xa[1:1 + HB * NJ].reshape(NJ, HB, 128, SB * 128)),
        "xtail": np.ascontiguousarray(xa[1 + HB * NJ:NB]),
    }


def kernel(x, w_ih1, w_hh1, b_ih1, b_hh1, w_ih2, w_hh2, b_ih2, b_hh2, w_out, b_out,
           _W=W_BURN):
    x = np.asarray(x, dtype=np.float32)
    B_, T_, _ = x.shape
    chunk = T_ // NC
    T_c = chunk + _W
    assert T_ % NC == 0 and T_c % SB == 0
    nc = _get_program(T_c)

    shared = _host_prep(w_ih1, w_hh1, b_ih1, b_hh1, w_ih2, w_hh2, b_ih2, b_hh2,
                        w_out, b_out)
    in_maps = []
    for c in range(NC):
        lo = 0 if c == 0 else c * chunk - _W
        xa = _prep_x(np.ascontiguousarray(x[:, lo:lo + T_c]))  # [NB, 128, 512]
        in_maps.append({**shared, **_split_x(xa)})

    res = bass_utils.run_bass_kernel_spmd(nc, in_maps, core_ids=list(range(NC)))
    y = np.empty((B_, T_, D_OUT), np.float32)
    for c in range(NC):
        yc = res.results[c]["y"][1:]  # [NB, 64, SB*128]; row b = y(4b..4b+3)
        NB = yc.shape[0]
        yc = yc.reshape(NB, 64, SB, 128).transpose(3, 0, 2, 1)  # [b, blk, s, d]
        yc = np.ascontiguousarray(yc).reshape(128, NB * SB, 64)
        if c == 0:
            y[:, 0:chunk] = yc[:, 0:chunk]
        else:
            y[:, c * chunk:(c + 1) * chunk] = yc[:, _W:]
    return y


# revision 9
# speedup vs baseline: 1.2095x; 1.2095x over previous
"""Trainium2 Bass kernel for a 2-layer LSTM regressor (B=128, T=4096, D=64, H=512).

Strategy (v3): sequence-parallel across the 8 cores (each core runs a 512-step
chunk with a 16-step burn-in prefix; the LSTM state contracts so initial-state
error decays to ~1e-11 by step 16). Full batch B=128 is the matmul moving
operand on every core.

v3 changes vs v2 (which measured 7.89 ms, ~97% PE-busy):
  - L2's 16 bias matmuls/step removed: the per-gate-unit bias is pre-written
    into the PSUM gate tiles by the DVE (tensor_copy of a pre-broadcast f32
    bias tile) and the gate matmuls run with start=False, accumulating onto it
    (has_written bits stay set after each PSUM buffer's first real group).
  - Gate quarters (i, f, g, o) in separate 1-bank PSUM tiles with per-quarter
    activations issued in dependency order, so the o-gate sigmoid + h-multiply
    are the only ops after the last gate matmul: recurrence chain tail drops
    from ~6 us to ~1.5 us and the per-step chain stalls disappear.
  - h1/h2 histories are 16-slot rings indexed (global_step % 16) - statically
    resolvable inside the hardware loop (32 steps/iteration) - removing the
    per-block carry copies from the critical path.
  - y emitted per 4-step block with FD=512 matmuls (4 instead of 16 per block)
    plus a DVE tensor_scalar bias add (bout matmul removed).
  - x DMAs for all 8 halves issued at the iteration top (bufs=10) so the SP
    engine prefetches a full iteration ahead; loop-head PE stalls (~8 us x 32)
    disappear.
"""
import sys
sys.path.insert(0, "/opt/trn_rl_repo")
import numpy as np
import ml_dtypes

import concourse.bass as bass
import concourse.bacc as bacc
import concourse.tile as tile
from concourse import mybir, bass_utils
from concourse.bass import ds, ts

F32 = mybir.dt.float32
BF16 = mybir.dt.bfloat16
AF = mybir.ActivationFunctionType
BF = ml_dtypes.bfloat16

B, T, D_IN = 128, 4096, 64
H, D_OUT = 512, 64
NC = 8
SB = 4           # steps per block
W_BURN = 16      # burn-in steps
UNROLL = 8       # L1/L2 block pairs ("halves") per loop iteration is 2*UNROLL
RING = 16        # h-history ring slots; must divide 4*2*UNROLL


# ---------------------------------------------------------------- host prep
def _prep_whh(w):
    """[2048, 512] -> [128, 16*4*128] bf16 lhsT tiles, col=(m*4+k)*128+j.
    Gate order is torch order (i, f, g, o): m0-3=i, m4-7=f, m8-11=g, m12-15=o."""
    t = w.reshape(16, 128, 4, 128)  # [m, j, k, p]
    t = t.transpose(3, 0, 2, 1)  # [p, m, k, j]
    return np.ascontiguousarray(t.reshape(128, 16 * 4 * 128)).astype(BF)


def _prep_wih1(w_ih1, b1):
    """[2048, 64] + bias -> [128, 2048] bf16 (rows 64/65 = bias hi/lo, 66+ zero)."""
    top = w_ih1.T  # [64, 2048]
    b = b1.astype(np.float32)
    bhi = b.astype(BF).astype(np.float32)
    blo = b - bhi
    pad = np.zeros((62, 2048), np.float32)
    return np.concatenate([top, bhi[None], blo[None], pad], 0).astype(BF)


def _prep_b2f(b2):
    """bias -> [128, 2048] f32 pre-broadcast: [p, m*128+c] = b2[m*128+p]."""
    v = b2.astype(np.float32).reshape(16, 128)  # [m, p]
    arr = np.broadcast_to(v.T[:, :, None], (128, 16, 128))
    return np.ascontiguousarray(arr.reshape(128, 2048))


def _prep_wout(w_out):
    """[64, 512] -> [128, 4*64]: [p, k*64+d] = w_out[d, k*128+p]."""
    t = w_out.reshape(64, 4, 128).transpose(2, 1, 0)  # [p, k, d]
    return np.ascontiguousarray(t.reshape(128, 256)).astype(BF)


def _prep_x(x_core):
    """[128, T_c, 64] -> blocks [NB, 128, 4*128] bf16 (col = s*128 + b).
    Rows 64/65 = 1.0 (bias), rows 66..127 = 0 (pad to full K=128)."""
    bc, T_c, _ = x_core.shape
    nb = T_c // SB
    arr = x_core.transpose(1, 2, 0).reshape(nb, SB, 64, bc)  # [blk, s, d, b]
    arr = arr.transpose(0, 2, 1, 3)  # [blk, d, s, b]
    ones = np.ones((nb, 2, SB, bc), np.float32)
    pad = np.zeros((nb, 62, SB, bc), np.float32)
    xa = np.concatenate([arr, ones, pad], 1).reshape(nb, 128, SB * bc)
    return xa.astype(BF)


# ---------------------------------------------------------------- program
def build_program(T_c, n_cores=NC):
    NB = T_c // SB
    HB = 2 * UNROLL                      # halves (block pairs) per iteration
    NJ = (NB - 2) // HB                  # loop iterations; L1 blocks 1..HB*NJ
    NTAIL = NB - 1 - HB * NJ             # L1 blocks unrolled in the epilogue
    assert T_c % SB == 0 and NTAIL >= 1 and (SB * HB) % RING == 0
    nc = bacc.Bacc("TRN2", target_bir_lowering=False, debug=False, num_devices=n_cores)

    d = {}
    d["w1hh"] = nc.dram_tensor("w1hh", [128, 8192], BF16, kind="ExternalInput")
    d["w1ih"] = nc.dram_tensor("w1ih", [128, 2048], BF16, kind="ExternalInput")
    d["w2hh"] = nc.dram_tensor("w2hh", [128, 8192], BF16, kind="ExternalInput")
    d["w2ih"] = nc.dram_tensor("w2ih", [128, 8192], BF16, kind="ExternalInput")
    d["b2f"] = nc.dram_tensor("b2f", [128, 2048], F32, kind="ExternalInput")
    d["wout"] = nc.dram_tensor("wout", [128, 256], BF16, kind="ExternalInput")
    d["boutv"] = nc.dram_tensor("boutv", [64, 1], F32, kind="ExternalInput")
    d["x0"] = nc.dram_tensor("x0", [128, SB * 128], BF16, kind="ExternalInput")
    d["xm"] = nc.dram_tensor("xm", [NJ, HB, 128, SB * 128], BF16, kind="ExternalInput")
    d["xtail"] = nc.dram_tensor("xtail", [NTAIL, 128, SB * 128], BF16, kind="ExternalInput")
    # row b+1 = y(4b..4b+3); row 0 is a dummy (pipeline warmup), dropped on host
    d["y"] = nc.dram_tensor("y", [NB + 1, 64, SB * 128], F32, kind="ExternalOutput")

    with tile.TileContext(nc) as tc:
        with tc.tile_pool(name="persist", bufs=1) as pp, \
             tc.tile_pool(name="work", bufs=2) as wp, \
             tc.tile_pool(name="xin", bufs=10) as xp_pool, \
             tc.tile_pool(name="psum", bufs=1, space="PSUM") as psp:

            w1hh = pp.tile([128, 8192], BF16)
            w1ih = pp.tile([128, 2048], BF16)
            w2hh = pp.tile([128, 8192], BF16)
            w2ih = pp.tile([128, 8192], BF16)
            b2f = pp.tile([128, 16, 128], F32)
            wout = pp.tile([128, 256], BF16)
            boutv = pp.tile([64, 1], F32)
            # order: everything the prologue L1 needs first (w1hh, w1ih, x0)
            nc.sync.dma_start(w1hh[:], d["w1hh"].ap())
            nc.sync.dma_start(w1ih[:], d["w1ih"].ap())
            xc0 = xp_pool.tile([128, SB * 128], BF16, tag="xc")
            nc.sync.dma_start(xc0[:], d["x0"].ap())
            for t_, dr in [(w2hh, "w2hh"), (w2ih, "w2ih"), (b2f, "b2f"),
                           (wout, "wout"), (boutv, "boutv")]:
                nc.sync.dma_start(t_[:], d[dr].ap())

            # h histories: ring of RING slots; step g reads slot g%RING,
            # writes (g+1)%RING. Zeros = initial state.
            H1 = pp.tile([128, 4, RING, 128], BF16)
            H2 = pp.tile([128, 4, RING, 128], BF16)
            c1 = pp.tile([128, 4, 128], F32)
            c2 = pp.tile([128, 4, 128], F32)
            nc.vector.memset(H1[:], 0.0)
            nc.vector.memset(H2[:], 0.0)
            nc.vector.memset(c1[:], 0.0)
            nc.vector.memset(c2[:], 0.0)

            def gates_and_state(qt, cst, hdst, lt):
                """Per-quarter activations + cell update. qt = [qi,qf,qg,qo].
                f stays f32 (its error is amplified by 1/(1-f) in the c fixpoint);
                i,g,o,tanh(c) are bf16 (same rounding class as the bf16 h)."""
                ai = wp.tile([128, 4, 128], BF16, tag=f"ai{lt}")
                af = wp.tile([128, 4, 128], F32, tag=f"af{lt}")
                ag = wp.tile([128, 4, 128], BF16, tag=f"ag{lt}")
                ao = wp.tile([128, 4, 128], BF16, tag=f"ao{lt}")
                tmp = wp.tile([128, 4, 128], BF16, tag=f"tmp{lt}")
                tct = wp.tile([128, 4, 128], BF16, tag=f"tct{lt}")
                nc.scalar.activation(ai[:], qt[0][:], AF.Sigmoid)
                nc.scalar.activation(af[:], qt[1][:], AF.Sigmoid)
                nc.scalar.activation(ag[:], qt[2][:], AF.Tanh)
                nc.scalar.activation(ao[:], qt[3][:], AF.Sigmoid)
                nc.vector.tensor_mul(tmp[:], ai[:], ag[:])
                nc.vector.tensor_mul(cst[:], af[:], cst[:])
                nc.vector.tensor_add(cst[:], cst[:], tmp[:])
                nc.scalar.activation(tct[:], cst[:], AF.Tanh)
                nc.vector.tensor_mul(hdst, ao[:], tct[:])

            def l2_alloc_fill():
                """Allocate + bias-fill the 4 L2 gate quarters (issued at step
                start so the DVE fills run ahead of the chain ops)."""
                qt = [psp.tile([128, 4, 128], F32, tag="g", bufs=6, name=f"g2_{q}")
                      for q in range(4)]
                for q in range(4):
                    nc.vector.tensor_copy(qt[q][:], b2f[:, q * 4:(q + 1) * 4, :])
                return qt

            def l1_step(xblk, s, sr, sw):
                xs = xblk[:, s * 128:(s + 1) * 128]
                qt = [psp.tile([128, 4, 128], F32, tag="g", bufs=6, name=f"g1_{q}")
                      for q in range(4)]
                # start=True clears has_written for the WHOLE bank, so only the
                # first matmul per tile starts; later j-regions overwrite via
                # cleared bits, then their hh matmuls accumulate. This also
                # leaves every element's bit set for the next (L2) user.
                for q in range(4):
                    for j in range(4):
                        m = q * 4 + j
                        o = qt[q][:, j, :]
                        nc.tensor.matmul(o, w1ih[:, m * 128:(m + 1) * 128], xs,
                                         start=(j == 0), stop=False)
                        for k in range(4):
                            nc.tensor.matmul(
                                o, w1hh[:, (m * 4 + k) * 128:(m * 4 + k + 1) * 128],
                                H1[:, k, sr, :], start=False, stop=(k == 3))
                gates_and_state(qt, c1, H1[:, :, sw, :], "1")

            def l2_step(qt, sh1, sr2, sw2):
                """Gate matmuls accumulate (start=False) onto the DVE-written
                bias; every PSUM buffer got a start=True L1 group in the
                prologue, so has_written is set and start=False accumulates."""
                for q in range(4):
                    for j in range(4):
                        m = q * 4 + j
                        o = qt[q][:, j, :]
                        for k in range(4):
                            nc.tensor.matmul(
                                o, w2ih[:, (m * 4 + k) * 128:(m * 4 + k + 1) * 128],
                                H1[:, k, sh1, :], start=False, stop=False)
                        for k in range(4):
                            nc.tensor.matmul(
                                o, w2hh[:, (m * 4 + k) * 128:(m * 4 + k + 1) * 128],
                                H2[:, k, sr2, :], start=False, stop=(k == 3))
                gates_and_state(qt, c2, H2[:, :, sw2, :], "2")

            def y_block(y_ap, a):
                """y for one 4-step block from H2 ring slots a..a+3 (mod RING)."""
                yp = psp.tile([64, SB, 128], F32, tag="y", bufs=2)
                n1 = min(SB, RING - a)
                for k in range(4):
                    wk = wout[:, k * 64:(k + 1) * 64]
                    # only the very first matmul starts (bank-wide clear); the
                    # wrapped region's k0 writes via cleared has_written bits
                    nc.tensor.matmul(yp[:, 0:n1, :], wk, H2[:, k, a:a + n1, :],
                                     start=(k == 0), stop=(k == 3))
                    if n1 < SB:
                        nc.tensor.matmul(yp[:, n1:SB, :], wk, H2[:, k, 0:SB - n1, :],
                                         start=False, stop=(k == 3))
                ys = wp.tile([64, SB, 128], F32, tag="ystage")
                nc.vector.tensor_scalar_add(ys[:], yp[:], boutv[:])
                nc.sync.dma_start(y_ap, ys[:])

            def half(xt, y_ap, g0, has_l1=True, has_l2=True):
                """One L1 block (base gstep g0) + the L2 block one behind it
                (base gstep g0-4) + the y block two behind (slots g0-7..g0-4)."""
                y_block(y_ap, (g0 - 7) % RING)
                for s in range(SB):
                    qt2 = l2_alloc_fill() if has_l2 else None
                    if has_l1:
                        l1_step(xt, s, (g0 + s) % RING, (g0 + s + 1) % RING)
                    if has_l2:
                        l2_step(qt2, (g0 - 4 + s + 1) % RING,
                                (g0 - 4 + s) % RING, (g0 - 4 + s + 1) % RING)

            # prologue: L1 block 0 (gsteps 0..3), initial state = ring zeros
            for s in range(SB):
                l1_step(xc0, s, s, s + 1)

            with tc.For_i(0, NJ, 1, hint_engines=(
                    mybir.EngineType.PE, mybir.EngineType.Activation,
                    mybir.EngineType.DVE, mybir.EngineType.SP)) as j:
                xts = []
                for h in range(HB):
                    xt = xp_pool.tile([128, SB * 128], BF16, tag="xc")
                    nc.sync.dma_start(xt[:], d["xm"].ap()[ds(j, 1)][0, h])
                    xts.append(xt)
                yrows = d["y"].ap()[ts(j, HB)]
                for h in range(HB):
                    # L1 block HB*j+1+h -> base gstep 4*(HB*j+1+h); 32j drops mod 16
                    half(xts[h], yrows[h], (4 + 4 * h) % RING)

            # epilogue: NTAIL tail halves + trailing L2 block + final 2 y blocks
            b1 = HB * NJ + 1            # first epilogue L1 block
            xtl = []
            for i in range(NTAIL):
                xt = xp_pool.tile([128, SB * 128], BF16, tag="xc")
                nc.sync.dma_start(xt[:], d["xtail"].ap()[i])
                xtl.append(xt)
            for i in range(NTAIL):
                half(xtl[i], d["y"].ap()[b1 + i - 1], (4 * (b1 + i)) % RING)
            # trailing L2 block (no L1): L1 block would be b1+NTAIL = NB
            g0 = (4 * (b1 + NTAIL)) % RING
            half(None, d["y"].ap()[b1 + NTAIL - 1], g0, has_l1=False)
            # final y block (L2 block NB-1, slots g0-3..g0)
            y_block(d["y"].ap()[NB], (g0 - 3) % RING)

    nc.compile()
    return nc


_CACHE = {}


def _get_program(T_c):
    if T_c not in _CACHE:
        _CACHE[T_c] = build_program(T_c)
    return _CACHE[T_c]


def _host_prep(w_ih1, w_hh1, b_ih1, b_hh1, w_ih2, w_hh2, b_ih2, b_hh2, w_out, b_out):
    return {
        "w1hh": _prep_whh(np.asarray(w_hh1)),
        "w1ih": _prep_wih1(np.asarray(w_ih1), np.asarray(b_ih1) + np.asarray(b_hh1)),
        "w2hh": _prep_whh(np.asarray(w_hh2)),
        "w2ih": _prep_whh(np.asarray(w_ih2)),
        "b2f": _prep_b2f(np.asarray(b_ih2) + np.asarray(b_hh2)),
        "wout": _prep_wout(np.asarray(w_out)),
        "boutv": np.ascontiguousarray(np.asarray(b_out, np.float32)[:, None]),
    }


def _split_x(xa):
    """Split per-block x array into x0/xm/xtail dram layouts."""
    NB = xa.shape[0]
    HB = 2 * UNROLL
    NJ = (NB - 2) // HB
    return {
        "x0": np.ascontiguousarray(xa[0]),
        "xm": np.ascontiguousarray(xa[1:1 + HB * NJ].reshape(NJ, HB, 128, SB * 128)),
        "xtail": np.ascontiguousarray(xa[1 + HB * NJ:NB]),
    }


def kernel(x, w_ih1, w_hh1, b_ih1, b_hh1, w_ih2, w_hh2, b_ih2, b_hh2, w_out, b_out,
           _W=W_BURN):
    x = np.asarray(x, dtype=np.float32)
    B_, T_, _ = x.shape
    chunk = T_ // NC
    T_c = chunk + _W
    assert T_ % NC == 0 and T_c % SB == 0
    nc = _get_program(T_c)

    shared = _host_prep(w_ih1, w_hh1, b_ih1, b_hh1, w_ih2, w_hh2, b_ih2, b_hh2,
                        w_out, b_out)
    in_maps = []
    for c in range(NC):
        lo = 0 if c == 0 else c * chunk - _W
        xa = _prep_x(np.ascontiguousarray(x[:, lo:lo + T_c]))  # [NB, 128, 512]
        in_maps.append({**shared, **_split_x(xa)})

    res = bass_utils.run_bass_kernel_spmd(nc, in_maps, core_ids=list(range(NC)))
    y = np.empty((B_, T_, D_OUT), np.float32)
    for c in range(NC):
        yc = res.results[c]["y"][1:]  # [NB, 64, SB*128]; row b = y(4b..4b+3)
        NB = yc.shape[0]
        yc = yc.reshape(NB, 64, SB, 128).transpose(3, 0, 2, 1)  # [b, blk, s, d]
        yc = np.ascontiguousarray(yc).reshape(128, NB * SB, 64)
        if c == 0:
            y[:, 0:chunk] = yc[:, 0:chunk]
        else:
            y[:, c * chunk:(c + 1) * chunk] = yc[:, _W:]
    return y
